# revision 22
# baseline (speedup 1.0000x reference)
"""Trainium2 Bass kernel for nn_AttentionBlock (B=2, M=2048, N=1024, H=16, d=64).

Sharding (8 cores): data-parallel over batch (2) x tensor-parallel over heads
(4 groups of 4 heads). Per core, for its batch b and heads h0..h0+3:

  QK^T = wqk^T @ x_b^T        transposed-feature layout, 2 heads per 128-row chunk
  V    = x_b @ wv             natural [seq, feat] layout, plus a ones column
                              per head that rides the PV matmul to produce the
                              softmax denominator for free
  per head pair: S^T chunks via K=64 matmuls (2 heads row-packed in the PE),
                 E = exp(S^T/sqrt(d)), ctx^T += [V|1]^T E per head (M=65)
  out^T += wp^T @ ctx^T       partial projection, DMA'd out as [1024, 2048] f16

v3 structure:
  * one flat software pipeline across ALL (pair, i-tile, j-chunk) attention
    chunks: scores for chunk t+1 are always issued before PV of chunk t --
    including across pair and i-tile boundaries -- so the PE never sits
    behind the exp latency
  * QKV projection of i-tile i+1 and output projection of i-tile i are
    spliced into the chunk stream at the boundary, filling the ACT-bound
    stretches with PE work
  * normalization split: an early PSUM->SBUF copy releases the ctx PSUM
    banks immediately; the reciprocal/broadcast chain runs a few chunks
    later, entirely off the critical path
  * causal narrowing: diagonal chunks only compute columns >= c*128
    (scores, exp, mask, PV)
  * fp16 output partials (host accumulates in fp32)

v4 additions:
  * K=128 zero-padded score matmuls: K<=64 matmuls stream at HALF rate
    (measured 924 vs 513 ns for the same moving columns), so the K side
    is stored per-head with the other head's 64 partitions zeroed; the
    full 128-partition Q chunk streams against it (zeros kill the cross
    term).  Costs one one-time memset, saves ~29us of PE time.
  * per-k-chunk xT DMA so the first QK matmul starts after 1/8 of the
    i-tile transfer

The matmul datapath runs in fp16 with fp32 PSUM accumulation and fp32
softmax/normalization: end-to-end error ~4e-4 relative.

Host-side: the v-bias folds exactly into an effective output bias (softmax
rows sum to 1), carried by the head-group-0 core of each batch; the 1/sqrt(d)
score scale is applied inside the exp activation; host sums the 4 head-group
partials per batch.
"""

import numpy as np

P = 128
B, M, N = 2, 2048, 1024
H, D = 16, 64
HPC = 4          # heads per core
NCORES = 8
KC = N // P      # 8 contraction chunks over the model dim
NI = M // 512    # 4 i-tiles (query dim)
NJ = M // P      # 16 j-chunks (key dim)
SCALE = 0.125    # 1/sqrt(D)

_CACHE = {}


def _build_bass():
    import concourse.bacc as bacc
    import concourse.mybir as mybir
    import concourse.tile as tile
    from contextlib import ExitStack

    F32 = mybir.dt.float32
    F16 = mybir.dt.float16
    EXP = mybir.ActivationFunctionType.Exp
    IDENT = mybir.ActivationFunctionType.Identity

    nc = bacc.Bacc("TRN2", debug=False)

    xT_d = nc.dram_tensor("xT", [N, M], F16, kind="ExternalInput")
    wqk_d = nc.dram_tensor("wqk", [N, 4 * P], F16, kind="ExternalInput")
    bqk_d = nc.dram_tensor("bqk", [4 * P], F32, kind="ExternalInput")
    wv_d = nc.dram_tensor("wv", [N, HPC * D], F16, kind="ExternalInput")
    wp_d = nc.dram_tensor("wp", [HPC * D, N], F16, kind="ExternalInput")
    bp_d = nc.dram_tensor("bp", [N], F32, kind="ExternalInput")
    # mask template [128, 2, 128]: upper-tri(1) block duplicated per head row
    maskt_d = nc.dram_tensor("maskt", [P, 2, P], F16, kind="ExternalInput")
    outT_d = nc.dram_tensor("outT", [N, M], F16, kind="ExternalOutput")

    with tile.TileContext(nc) as tc, ExitStack() as top:
        consts = top.enter_context(tc.tile_pool(name="consts", bufs=1))

        # --- weights / constants resident in SBUF ---
        # DMA issue order matters for startup latency: wqk + mask + bias
        # first (first QK matmul + first diagonal chunk), wv next, the
        # output-side weights (wp, bp) after ph1(0)'s xt DMA.
        wqk_sb = consts.tile([P, KC, 4 * P], F16)       # [128, 8, 512]
        wv_sb = consts.tile([P, KC, HPC * D], F16)      # [128, 8, 256]
        wp_sb = consts.tile([P, 2, N], F16)             # [128, 2, 1024]
        bqk_sb = consts.tile([P, 4], F32)
        bp_sb = consts.tile([P, KC], F32)
        maskt_sb = consts.tile([P, 2, P], F16)
        Q_sb = consts.tile([P, 2, M], F16)              # [128, 2, 2048]
        # K side, one zero-padded 128-chunk per head: chunk 2p+h holds head
        # h of pair p on its own 64 partitions, the other 64 stay zero
        K2_sb = consts.tile([P, 4, M], F16)             # [128, 4, 2048]
        V_sb = consts.tile([P, NJ, HPC, D + 1], F16)    # [128, 16, 4, 65]
        ctx_sb = consts.tile([P, 2, M], F16)            # [128, 2, 2048]

        # ones column (denominator rider) -- single-writer on DVE
        nc.vector.memset(V_sb[:, :, :, D:D + 1], 1.0)
        # zero halves of the per-head K chunks (never written again)
        nc.vector.memset(K2_sb[D:P, 0::2, :], 0.0)
        nc.vector.memset(K2_sb[0:D, 1::2, :], 0.0)

        with ExitStack() as body:
            xt_pool = body.enter_context(tc.tile_pool(name="xt", bufs=2))
            # ph1 QK/V + ph3 proj share one PSUM tag ring (2 banks)
            psmm = body.enter_context(tc.tile_pool(name="psmm", bufs=2, space="PSUM"))
            pss = body.enter_context(tc.tile_pool(name="pss", bufs=2, space="PSUM"))
            psctx = body.enter_context(tc.tile_pool(name="psctx", bufs=2, space="PSUM"))
            epool = body.enter_context(tc.tile_pool(name="epool", bufs=3))
            npool = body.enter_context(tc.tile_pool(name="npool", bufs=2))
            opool = body.enter_context(tc.tile_pool(name="opool", bufs=3))

            ctx_ps = {}   # (p, i) -> (ctxA, ctxB) PSUM tiles
            ctx_cp = {}   # (p, i) -> (cA, cB) SBUF fp32 copies
            xts = {}      # i -> xt tile
            ph1_ps = {}   # i -> in-flight psmm tile of the current unit

            def xt_prefetch(i):
                isl = slice(i * 512, (i + 1) * 512)
                xt = xt_pool.tile([P, KC, 512], F16, tag="xt", name=f"xt{i}")
                # i=0: per-k-chunk DMAs interleaved with the wqk chunks so the
                # first QK matmul starts after 1/8 of both transfers.  Later
                # i-tiles are prefetched ~20 chunks ahead, so two coarse DMAs
                # keep the Sync issue cost down.
                if i == 0:
                    for k in range(KC):
                        nc.sync.dma_start(
                            wqk_sb[:, k, :], wqk_d[k * P:(k + 1) * P, :]
                        )
                        nc.sync.dma_start(xt[:, k, :], xT_d[k * P:(k + 1) * P, isl])
                else:
                    h = KC // 2
                    nc.sync.dma_start(
                        xt[:, 0:h, :],
                        xT_d[0:h * P, isl].rearrange("(ko p) f -> p ko f", p=P),
                    )
                    nc.sync.dma_start(
                        xt[:, h:KC, :],
                        xT_d[h * P:KC * P, isl].rearrange("(ko p) f -> p ko f", p=P),
                    )
                if i == 0:
                    nc.sync.dma_start(maskt_sb[:], maskt_d[:])
                    nc.sync.dma_start(
                        bqk_sb[:], bqk_d[:].rearrange("(m p) -> p m", p=P)
                    )
                    nc.sync.dma_start(
                        wv_sb[:], wv_d[:].rearrange("(ko p) m -> p ko m", p=P)
                    )
                    nc.sync.dma_start(
                        wp_sb[:], wp_d[:].rearrange("(ko p) m -> p ko m", p=P)
                    )
                    nc.sync.dma_start(
                        bp_sb[:], bp_d[:].rearrange("(m p) -> p m", p=P)
                    )
                xts[i] = xt

            def qk_unit(i, m, half):
                """Half of a QK projection m-chunk: 4 of 8 k-chunk matmuls."""
                isl = slice(i * 512, (i + 1) * 512)
                xt = xts[i]
                if half == 0:
                    ph1_ps[i] = psmm.tile(
                        [P, 512], F32, tag="mm", name=f"qkps{i}_{m}"
                    )
                ps = ph1_ps[i]
                for k in range(4 * half, 4 * half + 4):
                    nc.tensor.matmul(
                        ps[:],
                        wqk_sb[:, k, m * P:(m + 1) * P],
                        xt[:, k, :],
                        start=(k == 0),
                        stop=(k == KC - 1),
                        skip_group_check=True,
                    )
                if half == 0:
                    return
                if m < 2:
                    # Q chunk for pair m, two heads row-packed
                    nc.vector.tensor_scalar_add(
                        Q_sb[:, m, isl], ps[:], bqk_sb[:, m:m + 1]
                    )
                else:
                    # K chunk for pair m-2: split the two heads into
                    # their zero-padded per-head chunks
                    p_ = m - 2
                    nc.vector.tensor_scalar_add(
                        K2_sb[0:D, 2 * p_, isl], ps[0:D, :],
                        bqk_sb[0:D, m:m + 1],
                    )
                    nc.vector.tensor_scalar_add(
                        K2_sb[D:P, 2 * p_ + 1, isl], ps[D:P, :],
                        bqk_sb[D:P, m:m + 1],
                    )

            def v_unit(i, jsub, half):
                """Half of a V projection jsub-chunk: 4 of 8 k-chunk matmuls."""
                xt = xts[i]
                jc = 4 * i + jsub
                if half == 0:
                    ph1_ps[i] = psmm.tile([P, 512], F32, tag="mm", name=f"ph1ps{i}")
                pv = ph1_ps[i]
                for k in range(4 * half, 4 * half + 4):
                    nc.tensor.matmul(
                        pv[:, 0:HPC * D],
                        xt[:, k, jsub * P:(jsub + 1) * P],
                        wv_sb[:, k, :],
                        start=(k == 0),
                        stop=(k == KC - 1),
                        skip_group_check=True,
                    )
                if half == 1:
                    nc.vector.tensor_copy(
                        V_sb[:, jc, :, 0:D],
                        pv[:, 0:HPC * D].rearrange("p (h d) -> p h d", h=HPC),
                    )

            def emit_scores(p, i, jc):
                c = jc - 4 * i
                o = max(c, 0) * P
                jsl = slice(jc * P, (jc + 1) * P)
                qap = Q_sb[:, p, i * 512 + o:(i + 1) * 512]
                s2 = pss.tile([P, 2, 512], F32, tag="s")
                # K=128 matmuls: the other head's 64 lhsT partitions are
                # zero, so the full-height Q chunk streams against each head
                nc.tensor.matmul(
                    s2[:, 0, o:512], K2_sb[:, 2 * p, jsl], qap,
                    start=True, stop=True,
                )
                nc.tensor.matmul(
                    s2[:, 1, o:512], K2_sb[:, 2 * p + 1, jsl], qap,
                    start=True, stop=True,
                )
                e2 = epool.tile([P, 2, 512], F16, tag="e")
                nc.scalar.activation(
                    e2[:, :, o:512], s2[:, :, o:512], EXP, scale=SCALE
                )
                if c >= 0:
                    # causal zeroing of the 128-wide diagonal block
                    nc.vector.tensor_mul(
                        e2[:, :, o:o + P], e2[:, :, o:o + P], maskt_sb[:]
                    )
                return e2

            def emit_pv(p, i, jc, e2):
                c = jc - 4 * i
                o = max(c, 0) * P
                njc = 4 * i + 4
                if jc == 0:
                    ctx_ps[(p, i)] = (
                        psctx.tile([D + 1, 512], F32, tag="ctx", name=f"ctxA_{p}_{i}"),
                        psctx.tile([D + 1, 512], F32, tag="ctx", name=f"ctxB_{p}_{i}"),
                    )
                ctxA, ctxB = ctx_ps[(p, i)]
                nc.tensor.matmul(
                    ctxA[:, o:512],
                    V_sb[:, jc, 2 * p, :],
                    e2[:, 0, o:512],
                    start=(jc == 0), stop=(jc == njc - 1),
                    skip_group_check=True,
                )
                nc.tensor.matmul(
                    ctxB[:, o:512],
                    V_sb[:, jc, 2 * p + 1, :],
                    e2[:, 1, o:512],
                    start=(jc == 0), stop=(jc == njc - 1),
                    skip_group_check=True,
                )

            def norm_copy(p, i):
                """Drain ctx PSUM into SBUF (frees the banks) + stage denoms."""
                ctxA, ctxB = ctx_ps.pop((p, i))
                cA = npool.tile([D + 1, 512], F32, tag="cA")
                cB = npool.tile([D + 1, 512], F32, tag="cB")
                nc.vector.tensor_copy(cA[:], ctxA[:])
                nc.vector.tensor_copy(cB[:], ctxB[:])
                d0A = npool.tile([1, 512], F32, tag="d0A")
                d0B = npool.tile([1, 512], F32, tag="d0B")
                # partition-shift row 64 -> partition 0 (engines can't shift)
                nc.sync.dma_start(d0A[:], cA[D:D + 1, :])
                nc.sync.dma_start(d0B[:], cB[D:D + 1, :])
                ctx_cp[(p, i)] = (cA, cB, d0A, d0B)

            def norm_fin(p, i):
                isl = slice(i * 512, (i + 1) * 512)
                cA, cB, d0A, d0B = ctx_cp.pop((p, i))
                invA = npool.tile([1, 512], F32, tag="invA")
                invB = npool.tile([1, 512], F32, tag="invB")
                nc.vector.reciprocal_approx_fast(invA[:], d0A[:])
                nc.vector.reciprocal_approx_fast(invB[:], d0B[:])
                bcA = npool.tile([D, 512], F32, tag="bcA")
                bcB = npool.tile([D, 512], F32, tag="bcB")
                nc.gpsimd.partition_broadcast(bcA[:], invA[:], channels=D)
                nc.gpsimd.partition_broadcast(bcB[:], invB[:], channels=D)
                # head A: lanes 0-63 all the way through
                nc.vector.tensor_mul(ctx_sb[0:D, p, isl], cA[0:D, :], bcA[:])
                # head B: normalize on lanes 0-63, then DMA-shift the
                # 64-row block up to partitions 64-127 of ctx_sb
                stB = npool.tile([D, 512], F16, tag="stB")
                nc.vector.tensor_mul(stB[:], cB[0:D, :], bcB[:])
                nc.sync.dma_start(ctx_sb[D:P, p, isl], stB[:])

            def o_unit(i, om):
                isl = slice(i * 512, (i + 1) * 512)
                ps = psmm.tile([P, 512], F32, tag="mm")
                for kc in range(2):
                    nc.tensor.matmul(
                        ps[:],
                        wp_sb[:, kc, om * P:(om + 1) * P],
                        ctx_sb[:, kc, isl],
                        start=(kc == 0), stop=(kc == 1),
                        skip_group_check=True,
                    )
                st = opool.tile([P, 512], F16, tag="st")
                nc.vector.tensor_scalar_add(st[:], ps[:], bp_sb[:, om:om + 1])
                # issue the store from the near-idle gpsimd queue: keeps the
                # Sync queue short so the normalization DMAs stay low-latency
                nc.gpsimd.dma_start(outT_d[om * P:(om + 1) * P, isl], st[:])

            # ---- token stream ----------------------------------------------
            # Flat pipeline with 1-chunk PV lookahead; projection work is
            # broken into ~0.4-0.9us micro-units and woven BETWEEN attention
            # chunks so the PE's slack under the ACT-paced exp stream absorbs
            # it instead of forming PE-only (ACT-idle) blocks.
            def ph1_units(i):
                us = []
                for m in (0, 2, 1, 3):  # pair-0 Q and K first
                    us += [("QK", i, m, 0), ("QK", i, m, 1)]
                for js in range(4):
                    us += [("V", i, js, 0), ("V", i, js, 1)]
                return us

            def weave(chunk_toks, units, skip=0):
                """Distribute units round-robin after the C tokens, leaving
                the first `skip` C tokens unit-free (dependency lead time)."""
                slots = [t for t in chunk_toks if t[0] == "C"][skip:]
                n = len(slots)
                per = [len(units) // n + (1 if x < len(units) % n else 0)
                       for x in range(n)]
                out, it, ci = [], iter(units), -skip
                for t in chunk_toks:
                    out.append(t)
                    if t[0] == "C":
                        if ci >= 0:
                            for _ in range(per[ci]):
                                out.append(next(it))
                        ci += 1
                return out

            # i=0 warmup: only the units chunk (0,0,0) needs, then start the
            # exp stream immediately; the rest of ph1(0) rides the chunks
            stream = [("XT", 0),
                      ("QK", 0, 0, 0), ("QK", 0, 0, 1),
                      ("QK", 0, 2, 0), ("QK", 0, 2, 1),
                      ("V", 0, 0, 0), ("V", 0, 0, 1)]
            p00 = [("C", 0, 0, jc) for jc in range(4)]
            v_rest = []
            for js in range(1, 4):
                v_rest += [("V", 0, js, 0), ("V", 0, js, 1)]
            stream += weave(p00, v_rest)
            stream += [("QK", 0, 1, 0), ("QK", 0, 1, 1),
                       ("QK", 0, 3, 0), ("QK", 0, 3, 1)]
            for i in range(NI):
                njc = 4 * i + 4
                if i > 0:
                    p0 = [("C", 0, i, jc) for jc in range(2, njc)]
                    # output projection of the previous i-tile rides pair 0,
                    # starting 2 chunks in so the pair-1 norm chain finishes
                    stream += weave(
                        p0, [("O", i - 1, om) for om in range(KC)],
                        skip=min(2, len(p0) - 1),
                    )
                p1c = [("C", 1, i, jc) for jc in range(njc)]
                p1 = [p1c[0], ("NC", 0, i), p1c[1], p1c[2], ("NF", 0, i)] + p1c[3:]
                if i + 1 < NI:
                    # prefetch + QKV projection of the next i-tile ride pair 1
                    stream += [("XT", i + 1)]
                    stream += weave(p1, ph1_units(i + 1))
                    stream += [("FLUSH",), ("NC", 1, i),
                               ("C", 0, i + 1, 0), ("C", 0, i + 1, 1),
                               ("NF", 1, i)]
                else:
                    stream += p1
                    stream += [("FLUSH",), ("NC", 1, i), ("NF", 1, i)]
                    stream += [("O", i, om) for om in range(KC)]

            pending = None  # (p, i, jc, e2)

            def flush():
                nonlocal pending
                if pending is not None:
                    emit_pv(*pending)
                    pending = None

            for tok in stream:
                kind = tok[0]
                if kind == "C":
                    _, p, i, jc = tok
                    e2 = emit_scores(p, i, jc)
                    flush()
                    pending = (p, i, jc, e2)
                elif kind == "FLUSH":
                    flush()
                elif kind == "NC":
                    norm_copy(tok[1], tok[2])
                elif kind == "NF":
                    norm_fin(tok[1], tok[2])
                elif kind == "XT":
                    xt_prefetch(tok[1])
                elif kind == "QK":
                    qk_unit(tok[1], tok[2], tok[3])
                elif kind == "V":
                    v_unit(tok[1], tok[2], tok[3])
                elif kind == "O":
                    o_unit(tok[1], tok[2])
            assert pending is None

    nc.finalize()
    return nc


def _prep_core_inputs(c, x, w_attn, w_proj, b_attn, b_proj):
    b = c // 4
    h0 = (c % 4) * HPC
    wq, wk, wv_all = w_attn[:, 0:N], w_attn[:, N:2 * N], w_attn[:, 2 * N:3 * N]
    bq, bk, bv_all = b_attn[0:N], b_attn[N:2 * N], b_attn[2 * N:3 * N]
    hs = lambda k: slice(h0 * D + k * D, h0 * D + (k + 2) * D)
    wqk = np.ascontiguousarray(np.concatenate(
        [wq[:, hs(0)], wq[:, hs(2)], wk[:, hs(0)], wk[:, hs(2)]], axis=1
    ), dtype=np.float16)
    bqk = np.concatenate(
        [bq[hs(0)], bq[hs(2)], bk[hs(0)], bk[hs(2)]]
    ).astype(np.float32)
    wv = np.ascontiguousarray(wv_all[:, h0 * D:(h0 + HPC) * D], dtype=np.float16)
    wp = np.ascontiguousarray(w_proj[h0 * D:(h0 + HPC) * D, :], dtype=np.float16)
    xT = np.ascontiguousarray(x[b].T.astype(np.float16))
    if c % 4 == 0:
        # v-bias folds into the output bias exactly (softmax rows sum to 1)
        bp = (b_proj + bv_all @ w_proj).astype(np.float32)
    else:
        bp = np.zeros_like(b_proj, dtype=np.float32)
    # mask template: upper-tri-with-diag(128), duplicated for the 2 packed heads
    tri = np.triu(np.ones((P, P), np.float16))
    maskt = np.stack([tri, tri], axis=1)
    return dict(xT=xT, wqk=wqk, bqk=bqk, wv=wv, wp=wp, bp=bp,
                maskt=np.ascontiguousarray(maskt))


def _get_runner():
    """Build (once) a cached jitted SPMD executor for the Bass module."""
    if "runner" in _CACHE:
        return _CACHE["runner"]

    import jax
    import concourse.mybir as mybir
    from concourse.bass2jax import (
        _bass_exec_p, install_neuronx_cc_hook, partition_id_tensor,
        shard_map, Mesh, PartitionSpec,
    )

    install_neuronx_cc_hook()
    nc = _CACHE["nc"]
    partition_name = nc.partition_id_tensor.name if nc.partition_id_tensor else None

    in_names, out_names, out_avals, zero_shapes = [], [], [], []
    for alloc in nc.m.functions[0].allocations:
        if not isinstance(alloc, mybir.MemoryLocationSet):
            continue
        name = alloc.memorylocations[0].name
        if alloc.kind == "ExternalInput":
            if name != partition_name:
                in_names.append(name)
        elif alloc.kind == "ExternalOutput":
            shape = tuple(alloc.tensor_shape)
            dtype = mybir.dt.np(alloc.dtype)
            out_names.append(name)
            out_avals.append(jax.core.ShapedArray(shape, dtype))
            zero_shapes.append((shape, dtype))
    n_params = len(in_names)
    all_in_names = in_names + out_names
    if partition_name is not None:
        all_in_names.append(partition_name)

    def _body(*args):
        operands = list(args)
        if partition_name is not None:
            operands.append(partition_id_tensor())
        outs = _bass_exec_p.bind(
            *operands,
            out_avals=tuple(out_avals),
            in_names=tuple(all_in_names),
            out_names=tuple(out_names),
            lowering_input_output_aliases=(),
            sim_require_finite=True,
            sim_require_nnan=True,
            nc=nc,
        )
        return tuple(outs)

    devices = jax.devices()[:NCORES]
    mesh = Mesh(np.asarray(devices), ("core",))
    n_outs = len(out_names)
    sharded = jax.jit(
        shard_map(
            _body, mesh=mesh,
            in_specs=(PartitionSpec("core"),) * (n_params + n_outs),
            out_specs=(PartitionSpec("core"),) * n_outs,
            check_rep=False,
        ),
        donate_argnums=tuple(range(n_params, n_params + n_outs)),
        keep_unused=True,
    )

    def runner(in_maps):
        concat_in = [
            np.concatenate([np.asarray(in_maps[c][nm]) for c in range(NCORES)], axis=0)
            for nm in in_names
        ]
        concat_zeros = [
            np.zeros((NCORES * sh[0], *sh[1:]), dt) for sh, dt in zero_shapes
        ]
        out_arrs = sharded(*concat_in, *concat_zeros)
        return [
            {
                nm: np.asarray(out_arrs[k]).reshape(NCORES, *out_avals[k].shape)[c]
                for k, nm in enumerate(out_names)
            }
            for c in range(NCORES)
        ]

    _CACHE["runner"] = runner
    return runner


def _gather(results):
    out = np.zeros((B, M, N), np.float32)
    for c in range(NCORES):
        out[c // 4] += results[c]["outT"].astype(np.float32).T
    return out


def _make_in_maps(inputs):
    x = np.asarray(inputs["x"], np.float32)
    w_attn = np.asarray(inputs["w_attn"], np.float32)
    w_proj = np.asarray(inputs["w_proj"], np.float32)
    b_attn = np.asarray(inputs["b_attn"], np.float32)
    b_proj = np.asarray(inputs["b_proj"], np.float32)
    return [
        _prep_core_inputs(c, x, w_attn, w_proj, b_attn, b_proj)
        for c in range(NCORES)
    ]


def run(inputs, trace=False):
    """Returns (full output [B, M, N], BassKernelResults-or-None)."""
    if "nc" not in _CACHE:
        _CACHE["nc"] = _build_bass()
    in_maps = _make_in_maps(inputs)
    if trace:
        from concourse import bass_utils
        res = bass_utils.run_bass_kernel_spmd(
            _CACHE["nc"], in_maps, core_ids=list(range(NCORES)), trace=True
        )
        return _gather(res.results), res
    results = _get_runner()(in_maps)
    return _gather(results), None


def kernel(**inputs) -> np.ndarray:
    out, _ = run(inputs, trace=False)
    return out
